# revision 17
# baseline (speedup 1.0000x reference)
"""Trainium2 Bass kernel for nn_AdjLeafGNN (encoder + kNN graph + 2-layer GCN).

Self-contained: hardcodes all shapes. Shards the batch of 1024 images over
8 NeuronCores (128 images/core), computes the CNN encoder data-parallel,
AllGathers embeddings, then computes distance/adjacency rows + GCN row-sharded.

Returns (emb, adj, dist, logits_cls, logits_spread) like the reference.
"""
import numpy as np

import concourse.bacc as bacc
import concourse.mybir as mybir
import concourse.tile as tile
from concourse.bass_utils import run_bass_kernel_spmd

dt = mybir.dt
AF = mybir.ActivationFunctionType
ALU = mybir.AluOpType

N_CORES = 8
N = 1024                    # batch / graph nodes
NL = N // N_CORES           # nodes per core = 128
G = 32                      # images per group
NGROUPS = NL // G           # 4
EPS_BIAS = 5e-11            # index tie-break bias (d2 units)

# ---------------------------------------------------------------------------
# device program
# ---------------------------------------------------------------------------


def build_nc():
    nc = bacc.Bacc("TRN2", target_bir_lowering=False, num_devices=N_CORES)

    t = {}

    def inp(name, shape):
        t[name] = nc.dram_tensor(name, shape, dt.float32, kind="ExternalInput")

    def outp(name, shape):
        t[name] = nc.dram_tensor(name, shape, dt.float32, kind="ExternalOutput")

    inp("im2col1", [27, NL * 256])
    inp("w1c", [27, 64])
    inp("b1c", [64, 1])
    inp("w2pk", [3, 128, 128])
    inp("w2e0", [3, 64, 128])
    inp("b2c", [128, 1])
    inp("awt", [4, 9, 128, 128])
    inp("abt", [4, 128, 1])
    inp("pwt", [512, 128])
    inp("pbt", [128, 1])
    inp("fcw", [128, 256])
    inp("fcb", [256, 1])
    inp("gw1", [256, 256])
    inp("gb1", [256, 1])
    inp("gw2", [256, 256])
    inp("gb2", [256, 1])
    inp("cwt", [256, 11])
    inp("cbt", [1, 11])
    inp("biasrow", [1, N])
    inp("ident", [128, 128])

    outp("emb_out", [NL, 256])
    outp("dist_out", [NL, N])
    outp("adj_out", [NL, N])
    outp("cls_out", [NL, 10])
    outp("spr_out", [NL, 1])

    with tile.TileContext(nc) as tc:
        _body(nc, tc, t)
    nc.compile()
    return nc


def _encoder(nc, tc, t, wp, pp, psA, gapT):
    """conv stack -> gapT [128ch, NL nodes]."""
    # ---- weights. Only w1/b1 are DMA'd before the first im2col slab;
    # everything else loads right after it (PE starts sooner) ----
    w1_sb = wp.tile([27, 64], dt.float32)
    nc.sync.dma_start(w1_sb[:], t["w1c"][:])
    b1_sb = wp.tile([64, 1], dt.float32)
    nc.sync.dma_start(b1_sb[:], t["b1c"][:])
    # conv2 packed weights: w2p[kx] = [ky1(64ch); ky2(64ch)] -> [128, 128],
    # w2e[kx] = ky0 [64, 128]
    w2p_sb = [wp.tile([128, 128], dt.float32, name=f"w2p_sb{i}") for i in range(3)]
    w2e_sb = [wp.tile([64, 128], dt.float32, name=f"w2e_sb{i}") for i in range(3)]
    b2_sb = wp.tile([128, 1], dt.float32)
    aw_sb = {}
    for b in range(4):
        for tap in (range(9) if b < 2 else [4]):
            aw_sb[(b, tap)] = wp.tile([128, 128], dt.float32,
                                      name=f"aw_sb{b}_{tap}")
    ab_sb = [wp.tile([128, 1], dt.float32, name=f"ab_sb{b}") for b in range(4)]
    pw_sb = [wp.tile([128, 128], dt.float32, name=f"pw_sb{b}") for b in range(4)]
    pb_sb = wp.tile([128, 1], dt.float32)

    def _load_weights():
        for i in range(3):
            nc.sync.dma_start(w2p_sb[i][:], t["w2pk"][i])
            nc.sync.dma_start(w2e_sb[i][:], t["w2e0"][i])
        nc.sync.dma_start(b2_sb[:], t["b2c"][:])
        for (b, tap), tl in aw_sb.items():
            nc.sync.dma_start(tl[:], t["awt"][b, tap])
        for b in range(4):
            nc.sync.dma_start(ab_sb[b][:], t["abt"][b])
            nc.sync.dma_start(pw_sb[b][:], t["pwt"][b * 128:(b + 1) * 128, :])
        nc.sync.dma_start(pb_sb[:], t["pbt"][:])

    with (
        tc.tile_pool(name="conv", bufs=1) as cp,
        tc.tile_pool(name="colp", bufs=2) as colp,
        tc.tile_pool(name="enc_evac", bufs=3) as ep,
    ):
        for g in range(NGROUPS):
            # conv1: 16 chunks of N=512; im2col slab loaded in 2 halves.
            # Output lands in h1 partitions 0-63; partitions 64-127 get a
            # y+1-shifted copy (SBUF-SBUF DMA) so conv2 can contract
            # (ky=1, ky=2) as one K=128 matmul.
            h1 = cp.tile([128, G * 256], dt.float32, tag="h1")
            for hh in range(2):
                col = colp.tile([27, G * 128], dt.float32, tag="col")
                off = g * G * 256 + hh * G * 128
                nc.sync.dma_start(col[:], t["im2col1"][:, off:off + G * 128])
                if g == 0 and hh == 0:
                    _load_weights()
                for ch in range(G * 128 // 512):
                    ps = psA.tile([128, 512], dt.float32, tag="cps")
                    nc.tensor.matmul(ps[0:64, :], w1_sb[:],
                                     col[:, ch * 512:(ch + 1) * 512],
                                     start=True, stop=True)
                    o = hh * G * 128 + ch * 512
                    nc.scalar.activation(h1[0:64, o:o + 512], ps[0:64, :],
                                         AF.Relu, bias=b1_sb[:, 0:1])
                    if o == 0:
                        nc.sync.dma_start(h1[64:128, 0:496], h1[0:64, 16:512])
                    else:
                        nc.sync.dma_start(h1[64:128, o - 16:o + 496],
                                          h1[0:64, o:o + 512])

            # conv2: stride 2, 16x16 -> 8x8. Taps (ky=1,ky=2) packed K=128
            # (h1 top = iy=2y, bottom = iy=2y+1); ky=0 tap K=64 (iy=2y-1).
            h2 = cp.tile([128, G * 64], dt.float32, tag="h2")
            h1v = h1.rearrange("c (i y x) -> c i y x", i=G, y=16, x=16)
            for ch in range(G // 8):  # chunks of 8 images, N=512
                ps = psA.tile([128, 512], dt.float32, tag="cps")
                psv = ps.rearrange("o (i y x) -> o i y x", i=8, y=8, x=8)
                i0 = ch * 8
                for kxi, kx in enumerate((1, 0, 2)):
                    x0 = 1 if kx == 0 else 0
                    xs = 2 * x0 + kx - 1
                    xsl = slice(xs, xs + 2 * (8 - x0) - 1, 2)
                    rhs = h1v[:, i0:i0 + 8, 0:15:2, xsl]
                    nc.tensor.matmul(psv[:, :, 0:8, x0:8], w2p_sb[kx][:],
                                     rhs, start=(kxi == 0), stop=False,
                                     skip_group_check=True)
                    rhs0 = h1v[0:64, i0:i0 + 8, 1:14:2, xsl]
                    nc.tensor.matmul(psv[:, :, 1:8, x0:8], w2e_sb[kx][:],
                                     rhs0, start=False, stop=(kxi == 2),
                                     skip_group_check=True)
                nc.scalar.activation(h2[:, ch * 512:(ch + 1) * 512], ps[:],
                                     AF.Relu, bias=b2_sb[:, 0:1])

            # aspp branches (d=12,18 reduce to 1x1: only center tap in-bounds)
            taps9 = [(1, 1)] + [(ky, kx) for ky in range(3) for kx in range(3)
                                if (ky, kx) != (1, 1)]
            h2v = h2.rearrange("c (i y x) -> c i y x", i=G, y=8, x=8)
            aouts = []
            for b, d in enumerate((1, 6, 12, 18)):
                ao = cp.tile([128, G * 64], dt.float32, tag=f"aspp{b}",
                             name=f"aspp{b}")
                taps = taps9 if b < 2 else [(1, 1)]
                for ch in range(G // 8):
                    ps = psA.tile([128, 512], dt.float32, tag="cps")
                    psv = ps.rearrange("o (i y x) -> o i y x", i=8, y=8, x=8)
                    i0 = ch * 8
                    for ti, (ky, kx) in enumerate(taps):
                        y0 = max(0, -d * (ky - 1))
                        y1 = min(8, 8 - d * (ky - 1))
                        x0 = max(0, -d * (kx - 1))
                        x1 = min(8, 8 - d * (kx - 1))
                        rhs = h2v[:, i0:i0 + 8,
                                  y0 + d * (ky - 1):y1 + d * (ky - 1),
                                  x0 + d * (kx - 1):x1 + d * (kx - 1)]
                        nc.tensor.matmul(psv[:, :, y0:y1, x0:x1],
                                         aw_sb[(b, ky * 3 + kx)][:], rhs,
                                         start=(ti == 0),
                                         stop=(ti == len(taps) - 1),
                                         skip_group_check=True)
                    nc.scalar.activation(ao[:, ch * 512:(ch + 1) * 512], ps[:],
                                         AF.Relu, bias=ab_sb[b][:, 0:1])
                aouts.append(ao)

            # proj 1x1 (K=512 over 4 branch tiles) + relu
            hp = cp.tile([128, G * 64], dt.float32, tag="hp")
            for ch in range(G // 8):
                ps = psA.tile([128, 512], dt.float32, tag="cps")
                for b in range(4):
                    nc.tensor.matmul(ps[:], pw_sb[b][:],
                                     aouts[b][:, ch * 512:(ch + 1) * 512],
                                     start=(b == 0), stop=(b == 3))
                nc.scalar.activation(hp[:, ch * 512:(ch + 1) * 512], ps[:],
                                     AF.Relu, bias=pb_sb[:, 0:1])

            # gap: mean over 64 spatial positions
            gsum = ep.tile([128, G], dt.float32, tag="gsum")
            nc.vector.tensor_reduce(gsum[:],
                                    hp.rearrange("c (i s) -> c i s", i=G, s=64),
                                    axis=mybir.AxisListType.X, op=ALU.add)
            nc.scalar.activation(gapT[:, g * G:(g + 1) * G], gsum[:], AF.Copy,
                                 scale=1.0 / 64.0)


def _body(nc, tc, t):
    with (
        tc.tile_pool(name="wp", bufs=1) as wp,
        tc.tile_pool(name="pp", bufs=1) as pp,
        tc.tile_pool(name="psA", bufs=4, space="PSUM") as psA,
        tc.tile_pool(name="psB", bufs=2, space="PSUM") as psB,
        tc.tile_pool(name="dram", bufs=1, space="DRAM") as dramp,
    ):
        ident_sb = wp.tile([128, 128], dt.float32)
        nc.sync.dma_start(ident_sb[:], t["ident"][:])
        ones_col = wp.tile([128, 1], dt.float32)
        nc.vector.memset(ones_col[:], 1.0)
        ones_row = wp.tile([1, 128], dt.float32)
        nc.vector.memset(ones_row[:], 1.0)

        gapT = pp.tile([128, NL], dt.float32)
        _encoder(nc, tc, t, wp, pp, psA, gapT)

        # ---- fc: embT [256, NL] = fcw.T @ gapT + fcb; emb output ----
        fcw_sb = wp.tile([128, 256], dt.float32)
        nc.sync.dma_start(fcw_sb[:], t["fcw"][:])
        fcb_sb = wp.tile([128, 2], dt.float32)
        nc.sync.dma_start(fcb_sb[:], t["fcb"].rearrange("(m p) o -> p (m o)", p=128))
        embT = [pp.tile([128, NL], dt.float32, name=f"embT{m}") for m in range(2)]
        emb_sb = pp.tile([NL, 256], dt.float32)
        for m in range(2):
            ps = psB.tile([128, NL], dt.float32, tag="gps")
            nc.tensor.matmul(ps[:], fcw_sb[:, m * 128:(m + 1) * 128], gapT[:],
                             start=True, stop=True)
            nc.scalar.activation(embT[m][:], ps[:], AF.Identity,
                                 bias=fcb_sb[:, m:m + 1])
            tp = psB.tile([128, 128], dt.float32, tag="gps")
            nc.tensor.transpose(tp[:], embT[m][:], ident_sb[:])
            nc.scalar.activation(emb_sb[:, m * 128:(m + 1) * 128], tp[:], AF.Copy)
        nc.sync.dma_start(t["emb_out"][:], emb_sb[:])

        # ---- AllGather embT ----
        ag_in = dramp.tile([256, NL], dt.float32)
        ag_out = dramp.tile([256 * N_CORES, NL], dt.float32, addr_space="Shared")
        for m in range(2):
            nc.sync.dma_start(ag_in[m * 128:(m + 1) * 128, :], embT[m][:])
        nc.gpsimd.collective_compute(
            "AllGather", ALU.bypass,
            replica_groups=[list(range(N_CORES))],
            ins=[ag_in[:]], outs=[ag_out[:]],
        )

        with (
            tc.tile_pool(name="sp", bufs=1) as sp,
            tc.tile_pool(name="g_evac", bufs=3) as ep,
        ):
            # embT_all as [128, 16*128]; free blocks b = 2r+k
            ebT = sp.tile([128, 16 * 128], dt.float32)
            nc.sync.dma_start(ebT[:], ag_out.rearrange("(b p) c -> p b c", p=128))
            ebTv = ebT.rearrange("p (r k c) -> p r k c", r=8, k=2, c=128)

            # Distances from CENTERED embeddings: e' = e - mean(e). d(i,j) is
            # shift-invariant, and centering removes the ~41x cancellation of
            # sq_i + sq_j - 2*dot (||mean||^2 dominates ||e||^2 here), so our
            # d2 is ~exact. The reference's own uncentered fp32 rounding
            # (~6e-7 d2-level) decides a couple of near-tie kNN boundaries;
            # those rows are irreducible coin flips for any implementation
            # that doesn't replicate Eigen's accumulation bit-for-bit.
            msum = ep.tile([128, 2], dt.float32, tag="msum")
            for k in range(2):
                nc.vector.tensor_reduce(msum[:, k:k + 1], ebTv[:, :, k, :],
                                        axis=mybir.AxisListType.XY, op=ALU.add)
            mean = sp.tile([128, 2], dt.float32)
            nc.scalar.activation(mean[:], msum[:], AF.Copy, scale=1.0 / float(N))

            ebC = sp.tile([128, 16 * 128], dt.float32)
            ebCv = ebC.rearrange("p (r k c) -> p r k c", r=8, k=2, c=128)
            for k in range(2):
                nc.vector.tensor_scalar(ebCv[:, :, k, :], ebTv[:, :, k, :],
                                        mean[:, k:k + 1], None, op0=ALU.subtract)
            esq = sp.tile([128, 16 * 128], dt.float32)
            nc.scalar.activation(esq[:], ebC[:], AF.Square)
            esqv = esq.rearrange("p (r k c) -> p r k c", r=8, k=2, c=128)

            # lhsT for d2: lhs_k = -2 * (embT_local - mean)
            lhs_k = [sp.tile([128, 128], dt.float32, name=f"lhs_k{k}")
                     for k in range(2)]
            for k in range(2):
                nc.vector.tensor_scalar(lhs_k[k][:], embT[k][:], mean[:, k:k + 1],
                                        None, op0=ALU.subtract)
                nc.vector.tensor_scalar(lhs_k[k][:], lhs_k[k][:], -2.0, None,
                                        op0=ALU.mult)
            # local sq row: sum_p (0.5*lhs)^2 == e'^2, via ones-lhsT matmul
            sql_ps = psB.tile([1, 128], dt.float32, tag="gps")
            for k in range(2):
                lsq = ep.tile([128, 128], dt.float32, tag="lsq")
                nc.scalar.activation(lsq[:], lhs_k[k][:], AF.Square, scale=0.5)
                nc.tensor.matmul(sql_ps[:], ones_col[:], lsq[:],
                                 start=(k == 0), stop=(k == 1))
            sql_sb = sp.tile([1, 128], dt.float32)
            nc.scalar.activation(sql_sb[:], sql_ps[:], AF.Copy)

            # sq_all row + ones row (all partition-0: engine APs need
            # 32-aligned base partitions, so no [2, N] stacked tiles)
            ones_N = sp.tile([1, N], dt.float32)
            nc.vector.memset(ones_N[:], 1.0)
            sq_row = sp.tile([1, N], dt.float32)
            for half in range(2):
                sq_ps = psB.tile([1, 512], dt.float32, tag="gps")
                for k in range(2):
                    nc.tensor.matmul(sq_ps[:], ones_col[:],
                                     esqv[:, 4 * half:4 * half + 4, k, :],
                                     start=(k == 0), stop=(k == 1))
                nc.scalar.activation(sq_row[0:1, half * 512:(half + 1) * 512],
                                     sq_ps[:], AF.Copy)

            br_sb = ep.tile([1, N], dt.float32, tag="brsb")
            nc.sync.dma_start(br_sb[:], t["biasrow"][:])

            # ---- d2 accumulation + dist + biased selection ----
            dist_sb = sp.tile([NL, N], dt.float32)
            adj_sb = sp.tile([NL, N], dt.float32)
            negd = sp.tile([NL, N], dt.float32)
            for half in range(2):
                cs = slice(half * 512, (half + 1) * 512)
                ps = psB.tile([128, 512], dt.float32, tag="gps")
                for k in range(2):
                    nc.tensor.matmul(ps[:], lhs_k[k][:],
                                     ebCv[:, 4 * half:4 * half + 4, k, :],
                                     start=(k == 0), stop=False,
                                     skip_group_check=True)
                nc.tensor.matmul(ps[:], ones_row[:], sq_row[0:1, cs],
                                 start=False, stop=False, skip_group_check=True)
                nc.tensor.matmul(ps[:], sql_sb[:], ones_N[0:1, cs],
                                 start=False, stop=True, skip_group_check=True)
                rl = ep.tile([128, 512], dt.float32, tag="rl")
                nc.scalar.activation(rl[:], ps[:], AF.Relu)
                nc.scalar.activation(dist_sb[:, cs], rl[:], AF.Sqrt)
                # tie-break bias accumulates AFTER the dist reads
                nc.tensor.matmul(ps[:], ones_row[:], br_sb[0:1, cs], start=False,
                                 stop=True, skip_group_check=True)
                nc.scalar.activation(negd[:, cs], ps[:], AF.Copy, scale=-1.0)
            nc.sync.dma_start(t["dist_out"][:], dist_sb[:])

            m8 = ep.tile([128, 8], dt.float32, tag="m8")
            nc.vector.max(m8[:], negd[:])
            nc.vector.tensor_scalar(adj_sb[:], negd[:], m8[:, 5:6], None,
                                    op0=ALU.is_ge)
            nc.sync.dma_start(t["adj_out"][:], adj_sb[:])

            # ---- adjT blocks via PE transpose ----
            adjT = [sp.tile([128, 128], dt.float32, name=f"adjT{r}")
                    for r in range(8)]
            for r in range(8):
                tp = psB.tile([128, 128], dt.float32, tag="gps")
                nc.tensor.transpose(tp[:], adj_sb[:, r * 128:(r + 1) * 128],
                                    ident_sb[:])
                nc.scalar.activation(adjT[r][:], tp[:], AF.Copy)

            # ---- GCN weights ----
            gw1_sb = [wp.tile([128, 256], dt.float32, name=f"gw1_sb{k}")
                      for k in range(2)]
            gw2_sb = [wp.tile([128, 256], dt.float32, name=f"gw2_sb{k}")
                      for k in range(2)]
            for k in range(2):
                nc.sync.dma_start(gw1_sb[k][:], t["gw1"][k * 128:(k + 1) * 128, :])
                nc.sync.dma_start(gw2_sb[k][:], t["gw2"][k * 128:(k + 1) * 128, :])
            gb1_sb = wp.tile([128, 2], dt.float32)
            nc.sync.dma_start(gb1_sb[:],
                              t["gb1"].rearrange("(m p) o -> p (m o)", p=128))
            gb2_sb = wp.tile([128, 2], dt.float32)
            nc.sync.dma_start(gb2_sb[:],
                              t["gb2"].rearrange("(m p) o -> p (m o)", p=128))

            # layer 1: hg1[r] = emb_all[block r] @ gw1 (node-major out)
            hg1 = [sp.tile([128, 256], dt.float32, name=f"hg1_{r}")
                   for r in range(8)]
            for r in range(8):
                ps = psB.tile([128, 256], dt.float32, tag="gps")
                for k in range(2):
                    nc.tensor.matmul(ps[:], ebTv[:, r, k, :], gw1_sb[k][:],
                                     start=(k == 0), stop=(k == 1))
                nc.scalar.activation(hg1[r][:], ps[:], AF.Copy)
            x1T = [sp.tile([128, 128], dt.float32, name=f"x1T{m}")
                   for m in range(2)]
            for m in range(2):
                ps = psB.tile([128, 128], dt.float32, tag="gps")
                for r in range(8):
                    nc.tensor.matmul(ps[:], hg1[r][:, m * 128:(m + 1) * 128],
                                     adjT[r][:], start=(r == 0), stop=(r == 7))
                nc.scalar.activation(x1T[m][:], ps[:], AF.Relu,
                                     bias=gb1_sb[:, m:m + 1])

            # AllGather x1T
            ag2_in = dramp.tile([256, NL], dt.float32)
            ag2_out = dramp.tile([256 * N_CORES, NL], dt.float32,
                                 addr_space="Shared")
            for m in range(2):
                nc.sync.dma_start(ag2_in[m * 128:(m + 1) * 128, :], x1T[m][:])
            nc.gpsimd.collective_compute(
                "AllGather", ALU.bypass,
                replica_groups=[list(range(N_CORES))],
                ins=[ag2_in[:]], outs=[ag2_out[:]],
            )
            x1a = sp.tile([128, 16 * 128], dt.float32)
            nc.sync.dma_start(x1a[:], ag2_out.rearrange("(b p) c -> p b c", p=128))
            x1av = x1a.rearrange("p (r k c) -> p r k c", r=8, k=2, c=128)

            # layer 2
            hg2 = [sp.tile([128, 256], dt.float32, name=f"hg2_{r}")
                   for r in range(8)]
            for r in range(8):
                ps = psB.tile([128, 256], dt.float32, tag="gps")
                for k in range(2):
                    nc.tensor.matmul(ps[:], x1av[:, r, k, :], gw2_sb[k][:],
                                     start=(k == 0), stop=(k == 1))
                nc.scalar.activation(hg2[r][:], ps[:], AF.Copy)
            x2T = [sp.tile([128, 128], dt.float32, name=f"x2T{m}")
                   for m in range(2)]
            for m in range(2):
                ps = psB.tile([128, 128], dt.float32, tag="gps")
                for r in range(8):
                    nc.tensor.matmul(ps[:], hg2[r][:, m * 128:(m + 1) * 128],
                                     adjT[r][:], start=(r == 0), stop=(r == 7))
                nc.scalar.activation(x2T[m][:], ps[:], AF.Identity,
                                     bias=gb2_sb[:, m:m + 1])

            # ---- heads ----
            cw_sb = [wp.tile([128, 11], dt.float32, name=f"cw_sb{k}")
                     for k in range(2)]
            for k in range(2):
                nc.sync.dma_start(cw_sb[k][:], t["cwt"][k * 128:(k + 1) * 128, :])
            cb_sb = wp.tile([1, 11], dt.float32)
            nc.sync.dma_start(cb_sb[:], t["cbt"][:])
            ps = psB.tile([128, 11], dt.float32, tag="gps")
            for k in range(2):
                nc.tensor.matmul(ps[:], x2T[k][:], cw_sb[k][:], start=(k == 0),
                                 stop=False, skip_group_check=True)
            nc.tensor.matmul(ps[:], ones_row[:], cb_sb[:], start=False, stop=True,
                             skip_group_check=True)
            logit_sb = ep.tile([128, 11], dt.float32, tag="logit")
            nc.scalar.activation(logit_sb[:], ps[:], AF.Copy)
            nc.sync.dma_start(t["cls_out"][:], logit_sb[:, 0:10])
            nc.sync.dma_start(t["spr_out"][:], logit_sb[:, 10:11])


# ---------------------------------------------------------------------------
# host side
# ---------------------------------------------------------------------------

_NC_CACHE = {}


def _get_nc():
    if "nc" not in _NC_CACHE:
        _NC_CACHE["nc"] = build_nc()
    return _NC_CACHE["nc"]


def _f32(x):
    return np.ascontiguousarray(x, dtype=np.float32)


def _w2t(w2):
    return _f32(np.asarray(w2).transpose(2, 3, 1, 0).reshape(9, 64, 128))


def _w2pk(w2):
    w = _w2t(w2)
    return _f32(np.stack([np.concatenate([w[3 + kx], w[6 + kx]], 0)
                          for kx in range(3)]))


def _w2e0(w2):
    w = _w2t(w2)
    return _f32(np.stack([w[kx] for kx in range(3)]))


def prep_in_maps(images, w1, b1, w2, b2, aspp_w, aspp_b, proj_w, proj_b,
                 fc_w, fc_b, gcn1_w, gcn1_b, gcn2_w, gcn2_b, cls_w, cls_b,
                 spr_w, spr_b):
    # conv1 im2col on host: [27(ky,kx,c), 1024, 256]
    xp = np.pad(_f32(images), ((0, 0), (0, 0), (1, 1), (1, 1)))
    s = xp.strides
    v = np.lib.stride_tricks.as_strided(
        xp, (N, 3, 3, 3, 16, 16), (s[0], s[1], s[2], s[3], 2 * s[2], 2 * s[3]))
    col = _f32(v.transpose(2, 3, 1, 0, 4, 5).reshape(27, N, 256))

    vsr = np.float32(1.0) / np.sqrt(np.float32(6.0))
    c6 = np.float32(vsr * vsr)

    shared = dict(
        w1c=_f32(np.asarray(w1).transpose(2, 3, 1, 0).reshape(27, 64)),
        b1c=_f32(b1).reshape(64, 1),
        w2pk=_w2pk(w2),
        w2e0=_w2e0(w2),
        b2c=_f32(b2).reshape(128, 1),
        awt=_f32(np.asarray(aspp_w).transpose(0, 3, 4, 2, 1).reshape(4, 9, 128, 128)),
        abt=_f32(aspp_b).reshape(4, 128, 1),
        pwt=_f32(np.asarray(proj_w)[:, :, 0, 0].T),
        pbt=_f32(proj_b).reshape(128, 1),
        fcw=_f32(fc_w),
        fcb=_f32(fc_b).reshape(256, 1),
        gw1=_f32(c6 * np.asarray(gcn1_w, dtype=np.float32)),
        gb1=_f32(gcn1_b).reshape(256, 1),
        gw2=_f32(c6 * np.asarray(gcn2_w, dtype=np.float32)),
        gb2=_f32(gcn2_b).reshape(256, 1),
        cwt=_f32(np.concatenate([np.asarray(cls_w), np.asarray(spr_w)], axis=1)),
        cbt=_f32(np.concatenate([np.asarray(cls_b), np.asarray(spr_b)]))[None, :],
        biasrow=_f32(np.arange(N) * EPS_BIAS)[None, :],
        ident=np.eye(128, dtype=np.float32),
    )
    in_maps = []
    for c in range(N_CORES):
        m = dict(shared)
        m["im2col1"] = _f32(col[:, c * NL:(c + 1) * NL, :].reshape(27, NL * 256))
        in_maps.append(m)
    return in_maps


def run_on_device(in_maps):
    nc = _get_nc()
    return run_bass_kernel_spmd(nc, in_maps, core_ids=list(range(N_CORES)))


def kernel(**inputs):
    in_maps = prep_in_maps(**inputs)
    res = run_on_device(in_maps)
    emb = np.concatenate([res.results[c]["emb_out"] for c in range(N_CORES)], 0)
    adj = np.concatenate([res.results[c]["adj_out"] for c in range(N_CORES)], 0)
    dist = np.concatenate([res.results[c]["dist_out"] for c in range(N_CORES)], 0)
    lc = np.concatenate([res.results[c]["cls_out"] for c in range(N_CORES)], 0)
    ls = np.concatenate([res.results[c]["spr_out"][:, 0] for c in range(N_CORES)], 0)
    return emb, adj, dist, lc, ls


# revision 19
# speedup vs baseline: 1.1432x; 1.1432x over previous
"""Trainium2 Bass kernel for nn_AdjLeafGNN (encoder + kNN graph + 2-layer GCN).

Self-contained: hardcodes all shapes. Shards the batch of 1024 images over
8 NeuronCores (128 images/core), computes the CNN encoder data-parallel,
AllGathers embeddings, then computes distance/adjacency rows + GCN row-sharded.

Returns (emb, adj, dist, logits_cls, logits_spread) like the reference.
"""
import numpy as np

import concourse.bacc as bacc
import concourse.mybir as mybir
import concourse.tile as tile
from concourse.bass_utils import run_bass_kernel_spmd

dt = mybir.dt
AF = mybir.ActivationFunctionType
ALU = mybir.AluOpType

N_CORES = 8
N = 1024                    # batch / graph nodes
NL = N // N_CORES           # nodes per core = 128
G = 32                      # images per group
NGROUPS = NL // G           # 4
EPS_BIAS = 5e-11            # index tie-break bias (d2 units)

# ---------------------------------------------------------------------------
# device program
# ---------------------------------------------------------------------------


def build_nc():
    nc = bacc.Bacc("TRN2", target_bir_lowering=False, num_devices=N_CORES)

    t = {}

    def inp(name, shape):
        t[name] = nc.dram_tensor(name, shape, dt.float32, kind="ExternalInput")

    def outp(name, shape):
        t[name] = nc.dram_tensor(name, shape, dt.float32, kind="ExternalOutput")

    inp("im2col1", [128, NL * 64])
    inp("w1c", [128, 64])
    inp("b1c", [64, 1])
    inp("w2pk", [3, 128, 128])
    inp("w2e0", [3, 64, 128])
    inp("b2c", [128, 1])
    inp("awt", [4, 9, 128, 128])
    inp("abt", [4, 128, 1])
    inp("pwt", [512, 128])
    inp("pbt", [128, 1])
    inp("fcw", [128, 256])
    inp("fcb", [256, 1])
    inp("gw1", [256, 256])
    inp("gb1", [256, 1])
    inp("gw2", [256, 256])
    inp("gb2", [256, 1])
    inp("cwt", [256, 11])
    inp("cbt", [1, 11])
    inp("biasrow", [1, N])
    inp("ident", [128, 128])

    outp("emb_out", [NL, 256])
    outp("dist_out", [NL, N])
    outp("adj_out", [NL, N])
    outp("cls_out", [NL, 10])
    outp("spr_out", [NL, 1])

    with tile.TileContext(nc) as tc:
        _body(nc, tc, t)
    nc.compile()
    return nc


def _encoder(nc, tc, t, wp, pp, psA, gapT):
    """conv stack -> gapT [128ch, NL nodes]."""
    # ---- weights. Only w1/b1 are DMA'd before the first im2col slab;
    # everything else loads right after it (PE starts sooner) ----
    w1_sb = wp.tile([128, 64], dt.float32)
    nc.sync.dma_start(w1_sb[:], t["w1c"][:])
    b1_sb = wp.tile([64, 1], dt.float32)
    nc.sync.dma_start(b1_sb[:], t["b1c"][:])
    # conv2 packed weights: w2p[kx] = [ky1(64ch); ky2(64ch)] -> [128, 128],
    # w2e[kx] = ky0 [64, 128]
    w2p_sb = [wp.tile([128, 128], dt.float32, name=f"w2p_sb{i}") for i in range(3)]
    w2e_sb = [wp.tile([64, 128], dt.float32, name=f"w2e_sb{i}") for i in range(3)]
    b2_sb = wp.tile([128, 1], dt.float32)
    aw_sb = {}
    for b in range(4):
        for tap in (range(9) if b < 2 else [4]):
            aw_sb[(b, tap)] = wp.tile([128, 128], dt.float32,
                                      name=f"aw_sb{b}_{tap}")
    ab_sb = [wp.tile([128, 1], dt.float32, name=f"ab_sb{b}") for b in range(4)]
    pw_sb = [wp.tile([128, 128], dt.float32, name=f"pw_sb{b}") for b in range(4)]
    pb_sb = wp.tile([128, 1], dt.float32)

    def _load_weights():
        for i in range(3):
            nc.sync.dma_start(w2p_sb[i][:], t["w2pk"][i])
            nc.sync.dma_start(w2e_sb[i][:], t["w2e0"][i])
        nc.sync.dma_start(b2_sb[:], t["b2c"][:])
        for (b, tap), tl in aw_sb.items():
            nc.sync.dma_start(tl[:], t["awt"][b, tap])
        for b in range(4):
            nc.sync.dma_start(ab_sb[b][:], t["abt"][b])
            nc.sync.dma_start(pw_sb[b][:], t["pwt"][b * 128:(b + 1) * 128, :])
        nc.sync.dma_start(pb_sb[:], t["pbt"][:])

    with (
        tc.tile_pool(name="conv", bufs=1) as cp,
        tc.tile_pool(name="colp", bufs=2) as colp,
        tc.tile_pool(name="enc_evac", bufs=3) as ep,
    ):
        for g in range(NGROUPS):
            # conv1: 16 chunks of N=512 per group, packed 4-at-a-time into
            # PE row-groups (K=27 uses only 27 of 128 partition rows; the
            # quad im2col layout puts chunk 4b+a at partitions 32a, so 4
            # matmuls with tile_position=(32a,0) run concurrently).
            # Output lands in h1 partitions 0-63; partitions 64-127 get a
            # y+1-shifted copy (SBUF-SBUF DMA) so conv2 can contract
            # (ky=1, ky=2) as one K=128 matmul.
            h1 = cp.tile([128, G * 256], dt.float32, tag="h1")
            col = colp.tile([128, G * 64], dt.float32, tag="col")
            nc.sync.dma_start(col[:], t["im2col1"][:, g * G * 64:(g + 1) * G * 64])
            if g == 0:
                _load_weights()
            for b in range(G * 256 // 2048):
                pss = [psA.tile([64, 512], dt.float32, tag="cps",
                                name=f"c1ps{a}") for a in range(4)]
                for a in range(4):
                    nc.tensor.matmul(pss[a][:], w1_sb[32 * a:32 * a + 27, :],
                                     col[32 * a:32 * a + 27,
                                         b * 512:(b + 1) * 512],
                                     start=True, stop=True,
                                     tile_position=(32 * a, 0))
                for a in range(4):
                    o = (4 * b + a) * 512
                    nc.scalar.activation(h1[0:64, o:o + 512], pss[a][:],
                                         AF.Relu, bias=b1_sb[:, 0:1])
                    if o == 0:
                        nc.sync.dma_start(h1[64:128, 0:496], h1[0:64, 16:512])
                    else:
                        nc.sync.dma_start(h1[64:128, o - 16:o + 496],
                                          h1[0:64, o:o + 512])

            # conv2: stride 2, 16x16 -> 8x8. Taps (ky=1,ky=2) packed K=128
            # (h1 top = iy=2y, bottom = iy=2y+1); ky=0 tap K=64 (iy=2y-1).
            h2 = cp.tile([128, G * 64], dt.float32, tag="h2")
            h1v = h1.rearrange("c (i y x) -> c i y x", i=G, y=16, x=16)
            for ch in range(G // 8):  # chunks of 8 images, N=512
                ps = psA.tile([128, 512], dt.float32, tag="cps")
                psv = ps.rearrange("o (i y x) -> o i y x", i=8, y=8, x=8)
                i0 = ch * 8
                for kxi, kx in enumerate((1, 0, 2)):
                    x0 = 1 if kx == 0 else 0
                    xs = 2 * x0 + kx - 1
                    xsl = slice(xs, xs + 2 * (8 - x0) - 1, 2)
                    rhs = h1v[:, i0:i0 + 8, 0:15:2, xsl]
                    nc.tensor.matmul(psv[:, :, 0:8, x0:8], w2p_sb[kx][:],
                                     rhs, start=(kxi == 0), stop=False,
                                     skip_group_check=True)
                    rhs0 = h1v[0:64, i0:i0 + 8, 1:14:2, xsl]
                    nc.tensor.matmul(psv[:, :, 1:8, x0:8], w2e_sb[kx][:],
                                     rhs0, start=False, stop=(kxi == 2),
                                     skip_group_check=True)
                nc.scalar.activation(h2[:, ch * 512:(ch + 1) * 512], ps[:],
                                     AF.Relu, bias=b2_sb[:, 0:1])

            # aspp branches (d=12,18 reduce to 1x1: only center tap in-bounds)
            taps9 = [(1, 1)] + [(ky, kx) for ky in range(3) for kx in range(3)
                                if (ky, kx) != (1, 1)]
            h2v = h2.rearrange("c (i y x) -> c i y x", i=G, y=8, x=8)
            aouts = []
            for b, d in enumerate((1, 6, 12, 18)):
                ao = cp.tile([128, G * 64], dt.float32, tag=f"aspp{b}",
                             name=f"aspp{b}")
                taps = taps9 if b < 2 else [(1, 1)]
                for ch in range(G // 8):
                    ps = psA.tile([128, 512], dt.float32, tag="cps")
                    psv = ps.rearrange("o (i y x) -> o i y x", i=8, y=8, x=8)
                    i0 = ch * 8
                    for ti, (ky, kx) in enumerate(taps):
                        y0 = max(0, -d * (ky - 1))
                        y1 = min(8, 8 - d * (ky - 1))
                        x0 = max(0, -d * (kx - 1))
                        x1 = min(8, 8 - d * (kx - 1))
                        rhs = h2v[:, i0:i0 + 8,
                                  y0 + d * (ky - 1):y1 + d * (ky - 1),
                                  x0 + d * (kx - 1):x1 + d * (kx - 1)]
                        nc.tensor.matmul(psv[:, :, y0:y1, x0:x1],
                                         aw_sb[(b, ky * 3 + kx)][:], rhs,
                                         start=(ti == 0),
                                         stop=(ti == len(taps) - 1),
                                         skip_group_check=True)
                    nc.scalar.activation(ao[:, ch * 512:(ch + 1) * 512], ps[:],
                                         AF.Relu, bias=ab_sb[b][:, 0:1])
                aouts.append(ao)

            # proj 1x1 (K=512 over 4 branch tiles) + relu
            hp = cp.tile([128, G * 64], dt.float32, tag="hp")
            for ch in range(G // 8):
                ps = psA.tile([128, 512], dt.float32, tag="cps")
                for b in range(4):
                    nc.tensor.matmul(ps[:], pw_sb[b][:],
                                     aouts[b][:, ch * 512:(ch + 1) * 512],
                                     start=(b == 0), stop=(b == 3))
                nc.scalar.activation(hp[:, ch * 512:(ch + 1) * 512], ps[:],
                                     AF.Relu, bias=pb_sb[:, 0:1])

            # gap: mean over 64 spatial positions
            gsum = ep.tile([128, G], dt.float32, tag="gsum")
            nc.vector.tensor_reduce(gsum[:],
                                    hp.rearrange("c (i s) -> c i s", i=G, s=64),
                                    axis=mybir.AxisListType.X, op=ALU.add)
            nc.scalar.activation(gapT[:, g * G:(g + 1) * G], gsum[:], AF.Copy,
                                 scale=1.0 / 64.0)


def _body(nc, tc, t):
    with (
        tc.tile_pool(name="wp", bufs=1) as wp,
        tc.tile_pool(name="pp", bufs=1) as pp,
        tc.tile_pool(name="psA", bufs=4, space="PSUM") as psA,
        tc.tile_pool(name="psB", bufs=2, space="PSUM") as psB,
        tc.tile_pool(name="dram", bufs=1, space="DRAM") as dramp,
    ):
        ident_sb = wp.tile([128, 128], dt.float32)
        nc.sync.dma_start(ident_sb[:], t["ident"][:])
        ones_col = wp.tile([128, 1], dt.float32)
        nc.vector.memset(ones_col[:], 1.0)
        ones_row = wp.tile([1, 128], dt.float32)
        nc.vector.memset(ones_row[:], 1.0)

        gapT = pp.tile([128, NL], dt.float32)
        _encoder(nc, tc, t, wp, pp, psA, gapT)

        # ---- fc: embT [256, NL] = fcw.T @ gapT + fcb; emb output ----
        fcw_sb = wp.tile([128, 256], dt.float32)
        nc.sync.dma_start(fcw_sb[:], t["fcw"][:])
        fcb_sb = wp.tile([128, 2], dt.float32)
        nc.sync.dma_start(fcb_sb[:], t["fcb"].rearrange("(m p) o -> p (m o)", p=128))
        embT = [pp.tile([128, NL], dt.float32, name=f"embT{m}") for m in range(2)]
        emb_sb = pp.tile([NL, 256], dt.float32)
        for m in range(2):
            ps = psB.tile([128, NL], dt.float32, tag="gps")
            nc.tensor.matmul(ps[:], fcw_sb[:, m * 128:(m + 1) * 128], gapT[:],
                             start=True, stop=True)
            nc.scalar.activation(embT[m][:], ps[:], AF.Identity,
                                 bias=fcb_sb[:, m:m + 1])
            tp = psB.tile([128, 128], dt.float32, tag="gps")
            nc.tensor.transpose(tp[:], embT[m][:], ident_sb[:])
            nc.scalar.activation(emb_sb[:, m * 128:(m + 1) * 128], tp[:], AF.Copy)
        nc.sync.dma_start(t["emb_out"][:], emb_sb[:])

        # ---- AllGather embT ----
        ag_in = dramp.tile([256, NL], dt.float32)
        ag_out = dramp.tile([256 * N_CORES, NL], dt.float32, addr_space="Shared")
        for m in range(2):
            nc.sync.dma_start(ag_in[m * 128:(m + 1) * 128, :], embT[m][:])
        nc.gpsimd.collective_compute(
            "AllGather", ALU.bypass,
            replica_groups=[list(range(N_CORES))],
            ins=[ag_in[:]], outs=[ag_out[:]],
        )

        with (
            tc.tile_pool(name="sp", bufs=1) as sp,
            tc.tile_pool(name="g_evac", bufs=3) as ep,
        ):
            # embT_all as [128, 16*128]; free blocks b = 2r+k
            ebT = sp.tile([128, 16 * 128], dt.float32)
            nc.sync.dma_start(ebT[:], ag_out.rearrange("(b p) c -> p b c", p=128))
            ebTv = ebT.rearrange("p (r k c) -> p r k c", r=8, k=2, c=128)

            # Distances from CENTERED embeddings: e' = e - mean(e). d(i,j) is
            # shift-invariant, and centering removes the ~41x cancellation of
            # sq_i + sq_j - 2*dot (||mean||^2 dominates ||e||^2 here), so our
            # d2 is ~exact. The reference's own uncentered fp32 rounding
            # (~6e-7 d2-level) decides a couple of near-tie kNN boundaries;
            # those rows are irreducible coin flips for any implementation
            # that doesn't replicate Eigen's accumulation bit-for-bit.
            msum = ep.tile([128, 2], dt.float32, tag="msum")
            for k in range(2):
                nc.vector.tensor_reduce(msum[:, k:k + 1], ebTv[:, :, k, :],
                                        axis=mybir.AxisListType.XY, op=ALU.add)
            mean = sp.tile([128, 2], dt.float32)
            nc.scalar.activation(mean[:], msum[:], AF.Copy, scale=1.0 / float(N))

            ebC = sp.tile([128, 16 * 128], dt.float32)
            ebCv = ebC.rearrange("p (r k c) -> p r k c", r=8, k=2, c=128)
            for k in range(2):
                nc.vector.tensor_scalar(ebCv[:, :, k, :], ebTv[:, :, k, :],
                                        mean[:, k:k + 1], None, op0=ALU.subtract)
            esq = sp.tile([128, 16 * 128], dt.float32)
            nc.scalar.activation(esq[:], ebC[:], AF.Square)
            esqv = esq.rearrange("p (r k c) -> p r k c", r=8, k=2, c=128)

            # lhsT for d2: lhs_k = -2 * (embT_local - mean)
            lhs_k = [sp.tile([128, 128], dt.float32, name=f"lhs_k{k}")
                     for k in range(2)]
            for k in range(2):
                nc.vector.tensor_scalar(lhs_k[k][:], embT[k][:], mean[:, k:k + 1],
                                        None, op0=ALU.subtract)
                nc.vector.tensor_scalar(lhs_k[k][:], lhs_k[k][:], -2.0, None,
                                        op0=ALU.mult)
            # local sq row: sum_p (0.5*lhs)^2 == e'^2, via ones-lhsT matmul
            sql_ps = psB.tile([1, 128], dt.float32, tag="gps")
            for k in range(2):
                lsq = ep.tile([128, 128], dt.float32, tag="lsq")
                nc.scalar.activation(lsq[:], lhs_k[k][:], AF.Square, scale=0.5)
                nc.tensor.matmul(sql_ps[:], ones_col[:], lsq[:],
                                 start=(k == 0), stop=(k == 1))
            sql_sb = sp.tile([1, 128], dt.float32)
            nc.scalar.activation(sql_sb[:], sql_ps[:], AF.Copy)

            # sq_all row + ones row (all partition-0: engine APs need
            # 32-aligned base partitions, so no [2, N] stacked tiles)
            ones_N = sp.tile([1, N], dt.float32)
            nc.vector.memset(ones_N[:], 1.0)
            sq_row = sp.tile([1, N], dt.float32)
            for half in range(2):
                sq_ps = psB.tile([1, 512], dt.float32, tag="gps")
                for k in range(2):
                    nc.tensor.matmul(sq_ps[:], ones_col[:],
                                     esqv[:, 4 * half:4 * half + 4, k, :],
                                     start=(k == 0), stop=(k == 1))
                nc.scalar.activation(sq_row[0:1, half * 512:(half + 1) * 512],
                                     sq_ps[:], AF.Copy)

            br_sb = ep.tile([1, N], dt.float32, tag="brsb")
            nc.sync.dma_start(br_sb[:], t["biasrow"][:])

            # ---- d2 accumulation + dist + biased selection ----
            dist_sb = sp.tile([NL, N], dt.float32)
            adj_sb = sp.tile([NL, N], dt.float32)
            negd = sp.tile([NL, N], dt.float32)
            for half in range(2):
                cs = slice(half * 512, (half + 1) * 512)
                ps = psB.tile([128, 512], dt.float32, tag="gps")
                for k in range(2):
                    nc.tensor.matmul(ps[:], lhs_k[k][:],
                                     ebCv[:, 4 * half:4 * half + 4, k, :],
                                     start=(k == 0), stop=False,
                                     skip_group_check=True)
                nc.tensor.matmul(ps[:], ones_row[:], sq_row[0:1, cs],
                                 start=False, stop=False, skip_group_check=True)
                nc.tensor.matmul(ps[:], sql_sb[:], ones_N[0:1, cs],
                                 start=False, stop=True, skip_group_check=True)
                rl = ep.tile([128, 512], dt.float32, tag="rl")
                nc.scalar.activation(rl[:], ps[:], AF.Relu)
                nc.scalar.activation(dist_sb[:, cs], rl[:], AF.Sqrt)
                # tie-break bias accumulates AFTER the dist reads
                nc.tensor.matmul(ps[:], ones_row[:], br_sb[0:1, cs], start=False,
                                 stop=True, skip_group_check=True)
                nc.scalar.activation(negd[:, cs], ps[:], AF.Copy, scale=-1.0)
            nc.sync.dma_start(t["dist_out"][:], dist_sb[:])

            m8 = ep.tile([128, 8], dt.float32, tag="m8")
            nc.vector.max(m8[:], negd[:])
            nc.vector.tensor_scalar(adj_sb[:], negd[:], m8[:, 5:6], None,
                                    op0=ALU.is_ge)
            nc.sync.dma_start(t["adj_out"][:], adj_sb[:])

            # ---- adjT blocks via PE transpose ----
            adjT = [sp.tile([128, 128], dt.float32, name=f"adjT{r}")
                    for r in range(8)]
            for r in range(8):
                tp = psB.tile([128, 128], dt.float32, tag="gps")
                nc.tensor.transpose(tp[:], adj_sb[:, r * 128:(r + 1) * 128],
                                    ident_sb[:])
                nc.scalar.activation(adjT[r][:], tp[:], AF.Copy)

            # ---- GCN weights ----
            gw1_sb = [wp.tile([128, 256], dt.float32, name=f"gw1_sb{k}")
                      for k in range(2)]
            gw2_sb = [wp.tile([128, 256], dt.float32, name=f"gw2_sb{k}")
                      for k in range(2)]
            for k in range(2):
                nc.sync.dma_start(gw1_sb[k][:], t["gw1"][k * 128:(k + 1) * 128, :])
                nc.sync.dma_start(gw2_sb[k][:], t["gw2"][k * 128:(k + 1) * 128, :])
            gb1_sb = wp.tile([128, 2], dt.float32)
            nc.sync.dma_start(gb1_sb[:],
                              t["gb1"].rearrange("(m p) o -> p (m o)", p=128))
            gb2_sb = wp.tile([128, 2], dt.float32)
            nc.sync.dma_start(gb2_sb[:],
                              t["gb2"].rearrange("(m p) o -> p (m o)", p=128))

            # layer 1: hg1[r] = emb_all[block r] @ gw1 (node-major out)
            hg1 = [sp.tile([128, 256], dt.float32, name=f"hg1_{r}")
                   for r in range(8)]
            for r in range(8):
                ps = psB.tile([128, 256], dt.float32, tag="gps")
                for k in range(2):
                    nc.tensor.matmul(ps[:], ebTv[:, r, k, :], gw1_sb[k][:],
                                     start=(k == 0), stop=(k == 1))
                nc.scalar.activation(hg1[r][:], ps[:], AF.Copy)
            x1T = [sp.tile([128, 128], dt.float32, name=f"x1T{m}")
                   for m in range(2)]
            for m in range(2):
                ps = psB.tile([128, 128], dt.float32, tag="gps")
                for r in range(8):
                    nc.tensor.matmul(ps[:], hg1[r][:, m * 128:(m + 1) * 128],
                                     adjT[r][:], start=(r == 0), stop=(r == 7))
                nc.scalar.activation(x1T[m][:], ps[:], AF.Relu,
                                     bias=gb1_sb[:, m:m + 1])

            # AllGather x1T
            ag2_in = dramp.tile([256, NL], dt.float32)
            ag2_out = dramp.tile([256 * N_CORES, NL], dt.float32,
                                 addr_space="Shared")
            for m in range(2):
                nc.sync.dma_start(ag2_in[m * 128:(m + 1) * 128, :], x1T[m][:])
            nc.gpsimd.collective_compute(
                "AllGather", ALU.bypass,
                replica_groups=[list(range(N_CORES))],
                ins=[ag2_in[:]], outs=[ag2_out[:]],
            )
            x1a = sp.tile([128, 16 * 128], dt.float32)
            nc.sync.dma_start(x1a[:], ag2_out.rearrange("(b p) c -> p b c", p=128))
            x1av = x1a.rearrange("p (r k c) -> p r k c", r=8, k=2, c=128)

            # layer 2
            hg2 = [sp.tile([128, 256], dt.float32, name=f"hg2_{r}")
                   for r in range(8)]
            for r in range(8):
                ps = psB.tile([128, 256], dt.float32, tag="gps")
                for k in range(2):
                    nc.tensor.matmul(ps[:], x1av[:, r, k, :], gw2_sb[k][:],
                                     start=(k == 0), stop=(k == 1))
                nc.scalar.activation(hg2[r][:], ps[:], AF.Copy)
            x2T = [sp.tile([128, 128], dt.float32, name=f"x2T{m}")
                   for m in range(2)]
            for m in range(2):
                ps = psB.tile([128, 128], dt.float32, tag="gps")
                for r in range(8):
                    nc.tensor.matmul(ps[:], hg2[r][:, m * 128:(m + 1) * 128],
                                     adjT[r][:], start=(r == 0), stop=(r == 7))
                nc.scalar.activation(x2T[m][:], ps[:], AF.Identity,
                                     bias=gb2_sb[:, m:m + 1])

            # ---- heads ----
            cw_sb = [wp.tile([128, 11], dt.float32, name=f"cw_sb{k}")
                     for k in range(2)]
            for k in range(2):
                nc.sync.dma_start(cw_sb[k][:], t["cwt"][k * 128:(k + 1) * 128, :])
            cb_sb = wp.tile([1, 11], dt.float32)
            nc.sync.dma_start(cb_sb[:], t["cbt"][:])
            ps = psB.tile([128, 11], dt.float32, tag="gps")
            for k in range(2):
                nc.tensor.matmul(ps[:], x2T[k][:], cw_sb[k][:], start=(k == 0),
                                 stop=False, skip_group_check=True)
            nc.tensor.matmul(ps[:], ones_row[:], cb_sb[:], start=False, stop=True,
                             skip_group_check=True)
            logit_sb = ep.tile([128, 11], dt.float32, tag="logit")
            nc.scalar.activation(logit_sb[:], ps[:], AF.Copy)
            nc.sync.dma_start(t["cls_out"][:], logit_sb[:, 0:10])
            nc.sync.dma_start(t["spr_out"][:], logit_sb[:, 10:11])


# ---------------------------------------------------------------------------
# host side
# ---------------------------------------------------------------------------

_NC_CACHE = {}


def _get_nc():
    if "nc" not in _NC_CACHE:
        _NC_CACHE["nc"] = build_nc()
    return _NC_CACHE["nc"]


def _f32(x):
    return np.ascontiguousarray(x, dtype=np.float32)


def _w1rep(w1):
    w = _f32(np.asarray(w1).transpose(2, 3, 1, 0).reshape(27, 64))
    out = np.zeros((128, 64), np.float32)
    for a in range(4):
        out[32 * a:32 * a + 27] = w
    return out


def _quad_im2col(col_core):
    # col_core [27, 64 chunks, 512] -> [128, 16, 512]: chunk 16*(j//4)+4*(j%4)+a
    out = np.zeros((128, 16, 512), np.float32)
    j = np.arange(16)
    for a in range(4):
        chunks = 16 * (j // 4) + 4 * (j % 4) + a
        out[32 * a:32 * a + 27] = col_core[:, chunks, :]
    return out.reshape(128, 8192)


def _w2t(w2):
    return _f32(np.asarray(w2).transpose(2, 3, 1, 0).reshape(9, 64, 128))


def _w2pk(w2):
    w = _w2t(w2)
    return _f32(np.stack([np.concatenate([w[3 + kx], w[6 + kx]], 0)
                          for kx in range(3)]))


def _w2e0(w2):
    w = _w2t(w2)
    return _f32(np.stack([w[kx] for kx in range(3)]))


def prep_in_maps(images, w1, b1, w2, b2, aspp_w, aspp_b, proj_w, proj_b,
                 fc_w, fc_b, gcn1_w, gcn1_b, gcn2_w, gcn2_b, cls_w, cls_b,
                 spr_w, spr_b):
    # conv1 im2col on host: [27(ky,kx,c), 1024, 256]
    xp = np.pad(_f32(images), ((0, 0), (0, 0), (1, 1), (1, 1)))
    s = xp.strides
    v = np.lib.stride_tricks.as_strided(
        xp, (N, 3, 3, 3, 16, 16), (s[0], s[1], s[2], s[3], 2 * s[2], 2 * s[3]))
    col = _f32(v.transpose(2, 3, 1, 0, 4, 5).reshape(27, N, 256))

    vsr = np.float32(1.0) / np.sqrt(np.float32(6.0))
    c6 = np.float32(vsr * vsr)

    shared = dict(
        w1c=_w1rep(w1),
        b1c=_f32(b1).reshape(64, 1),
        w2pk=_w2pk(w2),
        w2e0=_w2e0(w2),
        b2c=_f32(b2).reshape(128, 1),
        awt=_f32(np.asarray(aspp_w).transpose(0, 3, 4, 2, 1).reshape(4, 9, 128, 128)),
        abt=_f32(aspp_b).reshape(4, 128, 1),
        pwt=_f32(np.asarray(proj_w)[:, :, 0, 0].T),
        pbt=_f32(proj_b).reshape(128, 1),
        fcw=_f32(fc_w),
        fcb=_f32(fc_b).reshape(256, 1),
        gw1=_f32(c6 * np.asarray(gcn1_w, dtype=np.float32)),
        gb1=_f32(gcn1_b).reshape(256, 1),
        gw2=_f32(c6 * np.asarray(gcn2_w, dtype=np.float32)),
        gb2=_f32(gcn2_b).reshape(256, 1),
        cwt=_f32(np.concatenate([np.asarray(cls_w), np.asarray(spr_w)], axis=1)),
        cbt=_f32(np.concatenate([np.asarray(cls_b), np.asarray(spr_b)]))[None, :],
        biasrow=_f32(np.arange(N) * EPS_BIAS)[None, :],
        ident=np.eye(128, dtype=np.float32),
    )
    in_maps = []
    for c in range(N_CORES):
        m = dict(shared)
        m["im2col1"] = _quad_im2col(
            col[:, c * NL:(c + 1) * NL, :].reshape(27, 64, 512))
        in_maps.append(m)
    return in_maps


def run_on_device(in_maps):
    nc = _get_nc()
    return run_bass_kernel_spmd(nc, in_maps, core_ids=list(range(N_CORES)))


def kernel(**inputs):
    in_maps = prep_in_maps(**inputs)
    res = run_on_device(in_maps)
    emb = np.concatenate([res.results[c]["emb_out"] for c in range(N_CORES)], 0)
    adj = np.concatenate([res.results[c]["adj_out"] for c in range(N_CORES)], 0)
    dist = np.concatenate([res.results[c]["dist_out"] for c in range(N_CORES)], 0)
    lc = np.concatenate([res.results[c]["cls_out"] for c in range(N_CORES)], 0)
    ls = np.concatenate([res.results[c]["spr_out"][:, 0] for c in range(N_CORES)], 0)
    return emb, adj, dist, lc, ls


# revision 23
# speedup vs baseline: 146.4672x; 128.1168x over previous
"""Trainium2 Bass kernel for nn_AdjLeafGNN (encoder + kNN graph + 2-layer GCN).

Self-contained: hardcodes all shapes. Shards the batch of 1024 images over
8 NeuronCores (128 images/core), computes the CNN encoder data-parallel,
AllGathers embeddings, then computes distance/adjacency rows + GCN row-sharded.

Returns (emb, adj, dist, logits_cls, logits_spread) like the reference.
"""
import numpy as np

import concourse.bacc as bacc
import concourse.mybir as mybir
import concourse.tile as tile
from concourse.bass_utils import run_bass_kernel_spmd

dt = mybir.dt
AF = mybir.ActivationFunctionType
ALU = mybir.AluOpType

N_CORES = 8
N = 1024                    # batch / graph nodes
NL = N // N_CORES           # nodes per core = 128
G = 32                      # images per group
NGROUPS = NL // G           # 4
EPS_BIAS = 5e-11            # index tie-break bias (d2 units)

# ---------------------------------------------------------------------------
# device program
# ---------------------------------------------------------------------------


def build_nc():
    nc = bacc.Bacc("TRN2", target_bir_lowering=False, num_devices=N_CORES)

    t = {}

    def inp(name, shape):
        t[name] = nc.dram_tensor(name, shape, dt.float32, kind="ExternalInput")

    def outp(name, shape):
        t[name] = nc.dram_tensor(name, shape, dt.float32, kind="ExternalOutput")

    inp("im2col1", [128, NL * 64])
    inp("w1c", [128, 64])
    inp("b1c", [64, 1])
    inp("w2pk", [3, 128, 128])
    inp("w2e0", [3, 64, 128])
    inp("b2c", [128, 1])
    inp("awt", [4, 9, 128, 128])
    inp("abt", [4, 128, 1])
    inp("pwt", [512, 128])
    inp("pbt", [128, 1])
    inp("fcw", [128, 256])
    inp("fcb", [256, 1])
    inp("gw1", [256, 256])
    inp("gb1", [256, 1])
    inp("gw2", [256, 256])
    inp("gb2", [256, 1])
    inp("cwt", [256, 11])
    inp("cbt", [1, 11])
    inp("biasrow", [1, N])
    inp("ident", [128, 128])

    outp("emb_out", [NL, 256])
    outp("dist_out", [NL, N])
    outp("adj_out", [NL, N])
    outp("cls_out", [NL, 10])
    outp("spr_out", [NL, 1])

    with tile.TileContext(nc) as tc:
        _body(nc, tc, t)
    nc.compile()
    return nc


def _encoder(nc, tc, t, wp, pp, psA, gapT):
    """conv stack -> gapT [128ch, NL nodes]."""
    # ---- weights. Only w1/b1 are DMA'd before the first im2col slab;
    # everything else loads right after it (PE starts sooner) ----
    w1_sb = wp.tile([128, 64], dt.float32)
    nc.sync.dma_start(w1_sb[:], t["w1c"][:])
    b1_sb = wp.tile([64, 1], dt.float32)
    nc.sync.dma_start(b1_sb[:], t["b1c"][:])
    # conv2 packed weights: w2p[kx] = [ky1(64ch); ky2(64ch)] -> [128, 128],
    # w2e[kx] = ky0 [64, 128]
    w2p_sb = [wp.tile([128, 128], dt.float32, name=f"w2p_sb{i}") for i in range(3)]
    w2e_sb = [wp.tile([64, 128], dt.float32, name=f"w2e_sb{i}") for i in range(3)]
    w2eb_sb = [wp.tile([128, 128], dt.float32, name=f"w2eb_sb{i}")
               for i in range(3)]
    b2_sb = wp.tile([128, 1], dt.float32)
    aw_sb = {}
    for b in range(4):
        for tap in (range(9) if b < 2 else [4]):
            aw_sb[(b, tap)] = wp.tile([128, 128], dt.float32,
                                      name=f"aw_sb{b}_{tap}")
    ab_sb = [wp.tile([128, 1], dt.float32, name=f"ab_sb{b}") for b in range(4)]
    pw_sb = [wp.tile([128, 128], dt.float32, name=f"pw_sb{b}") for b in range(4)]
    pb_sb = wp.tile([128, 1], dt.float32)

    def _load_weights():
        for i in range(3):
            nc.sync.dma_start(w2p_sb[i][:], t["w2pk"][i])
            nc.sync.dma_start(w2e_sb[i][:], t["w2e0"][i])
            nc.sync.dma_start(w2eb_sb[i][64:128, :], t["w2e0"][i])
        nc.sync.dma_start(b2_sb[:], t["b2c"][:])
        for (b, tap), tl in aw_sb.items():
            nc.sync.dma_start(tl[:], t["awt"][b, tap])
        for b in range(4):
            nc.sync.dma_start(ab_sb[b][:], t["abt"][b])
            nc.sync.dma_start(pw_sb[b][:], t["pwt"][b * 128:(b + 1) * 128, :])
        nc.sync.dma_start(pb_sb[:], t["pbt"][:])

    with (
        tc.tile_pool(name="conv", bufs=1) as cp,
        tc.tile_pool(name="colp", bufs=2) as colp,
        tc.tile_pool(name="enc_evac", bufs=3) as ep,
    ):
        for g in range(NGROUPS):
            # conv1: 16 chunks of N=512 per group, packed 4-at-a-time into
            # PE row-groups (K=27 uses only 27 of 128 partition rows; the
            # quad im2col layout puts chunk 4b+a at partitions 32a, so 4
            # matmuls with tile_position=(32a,0) run concurrently).
            # Output lands in h1 partitions 0-63; partitions 64-127 get a
            # y+1-shifted copy (SBUF-SBUF DMA) so conv2 can contract
            # (ky=1, ky=2) as one K=128 matmul.
            h1 = cp.tile([128, G * 256], dt.float32, tag="h1")
            col = colp.tile([128, G * 64], dt.float32, tag="col")
            # per-b-block loads so the first matmul burst starts after 256KB,
            # not after the full 1MB slab
            for b in range(4):
                nc.sync.dma_start(
                    col[:, b * 512:(b + 1) * 512],
                    t["im2col1"][:, g * G * 64 + b * 512:g * G * 64 + (b + 1) * 512])
                if g == 0 and b == 0:
                    _load_weights()
            for b in range(G * 256 // 2048):
                pss = [psA.tile([64, 512], dt.float32, tag="cps",
                                name=f"c1ps{a}") for a in range(4)]
                for a in range(4):
                    nc.tensor.matmul(pss[a][:], w1_sb[32 * a:32 * a + 27, :],
                                     col[32 * a:32 * a + 27,
                                         b * 512:(b + 1) * 512],
                                     start=True, stop=True,
                                     tile_position=(32 * a, 0))
                for a in range(4):
                    o = (4 * b + a) * 512
                    nc.scalar.activation(h1[0:64, o:o + 512], pss[a][:],
                                         AF.Relu, bias=b1_sb[:, 0:1])
                    if o == 0:
                        nc.sync.dma_start(h1[64:128, 0:496], h1[0:64, 16:512])
                    else:
                        nc.sync.dma_start(h1[64:128, o - 16:o + 496],
                                          h1[0:64, o:o + 512])

            # conv2: stride 2, 16x16 -> 8x8. Taps (ky=1,ky=2) packed K=128
            # (h1 top = iy=2y, bottom = iy=2y+1); ky=0 tap K=64 (iy=2y-1).
            # Chunks are processed in pairs so the two K=64 ky0 taps run in
            # concurrent PE row-groups: chunk A reads h1[0:64] (iy=2y-1),
            # chunk B reads the shifted copy h1[64:128] at y-base 2y-2
            # (h1b[p]=h1[p+16] => same iy=2y-1 values).
            h2 = cp.tile([128, G * 64], dt.float32, tag="h2")
            h1v = h1.rearrange("c (i y x) -> c i y x", i=G, y=16, x=16)
            for pr in range(G // 16):  # pairs of 8-image chunks
                pss = [psA.tile([128, 512], dt.float32, tag="cps",
                                name=f"c2ps{q}") for q in range(2)]
                psvs = [p.rearrange("o (i y x) -> o i y x", i=8, y=8, x=8)
                        for p in pss]
                i0s = [pr * 16, pr * 16 + 8]
                for kxi, kx in enumerate((1, 0, 2)):
                    x0 = 1 if kx == 0 else 0
                    xs = 2 * x0 + kx - 1
                    xsl = slice(xs, xs + 2 * (8 - x0) - 1, 2)
                    for q in range(2):
                        rhs = h1v[:, i0s[q]:i0s[q] + 8, 0:15:2, xsl]
                        nc.tensor.matmul(psvs[q][:, :, 0:8, x0:8],
                                         w2p_sb[kx][:], rhs,
                                         start=(kxi == 0), stop=False,
                                         skip_group_check=True)
                for kxi, kx in enumerate((1, 0, 2)):
                    x0 = 1 if kx == 0 else 0
                    xs = 2 * x0 + kx - 1
                    xsl = slice(xs, xs + 2 * (8 - x0) - 1, 2)
                    rhs0 = h1v[0:64, i0s[0]:i0s[0] + 8, 1:14:2, xsl]
                    nc.tensor.matmul(psvs[0][:, :, 1:8, x0:8], w2e_sb[kx][:],
                                     rhs0, start=False, stop=(kxi == 2),
                                     tile_position=(0, 0),
                                     skip_group_check=True)
                    rhs1 = h1v[64:128, i0s[1]:i0s[1] + 8, 0:13:2, xsl]
                    nc.tensor.matmul(psvs[1][:, :, 1:8, x0:8], w2eb_sb[kx][64:128, :],
                                     rhs1, start=False, stop=(kxi == 2),
                                     tile_position=(64, 0),
                                     skip_group_check=True)
                for q in range(2):
                    cs2 = slice((pr * 2 + q) * 512, (pr * 2 + q + 1) * 512)
                    nc.scalar.activation(h2[:, cs2], pss[q][:],
                                         AF.Relu, bias=b2_sb[:, 0:1])

            # aspp branches (d=12,18 reduce to 1x1: only center tap in-bounds)
            taps9 = [(1, 1)] + [(ky, kx) for ky in range(3) for kx in range(3)
                                if (ky, kx) != (1, 1)]
            h2v = h2.rearrange("c (i y x) -> c i y x", i=G, y=8, x=8)
            aouts = []
            for b, d in enumerate((1, 6, 12, 18)):
                ao = cp.tile([128, G * 64], dt.float32, tag=f"aspp{b}",
                             name=f"aspp{b}")
                taps = taps9 if b < 2 else [(1, 1)]
                for ch in range(G // 8):
                    ps = psA.tile([128, 512], dt.float32, tag="cps")
                    psv = ps.rearrange("o (i y x) -> o i y x", i=8, y=8, x=8)
                    i0 = ch * 8
                    for ti, (ky, kx) in enumerate(taps):
                        y0 = max(0, -d * (ky - 1))
                        y1 = min(8, 8 - d * (ky - 1))
                        x0 = max(0, -d * (kx - 1))
                        x1 = min(8, 8 - d * (kx - 1))
                        rhs = h2v[:, i0:i0 + 8,
                                  y0 + d * (ky - 1):y1 + d * (ky - 1),
                                  x0 + d * (kx - 1):x1 + d * (kx - 1)]
                        nc.tensor.matmul(psv[:, :, y0:y1, x0:x1],
                                         aw_sb[(b, ky * 3 + kx)][:], rhs,
                                         start=(ti == 0),
                                         stop=(ti == len(taps) - 1),
                                         skip_group_check=True)
                    nc.scalar.activation(ao[:, ch * 512:(ch + 1) * 512], ps[:],
                                         AF.Relu, bias=ab_sb[b][:, 0:1])
                aouts.append(ao)

            # proj 1x1 (K=512 over 4 branch tiles) + relu
            hp = cp.tile([128, G * 64], dt.float32, tag="hp")
            for ch in range(G // 8):
                ps = psA.tile([128, 512], dt.float32, tag="cps")
                for b in range(4):
                    nc.tensor.matmul(ps[:], pw_sb[b][:],
                                     aouts[b][:, ch * 512:(ch + 1) * 512],
                                     start=(b == 0), stop=(b == 3))
                nc.scalar.activation(hp[:, ch * 512:(ch + 1) * 512], ps[:],
                                     AF.Relu, bias=pb_sb[:, 0:1])

            # gap: mean over 64 spatial positions
            gsum = ep.tile([128, G], dt.float32, tag="gsum")
            nc.vector.tensor_reduce(gsum[:],
                                    hp.rearrange("c (i s) -> c i s", i=G, s=64),
                                    axis=mybir.AxisListType.X, op=ALU.add)
            nc.scalar.activation(gapT[:, g * G:(g + 1) * G], gsum[:], AF.Copy,
                                 scale=1.0 / 64.0)


def _body(nc, tc, t):
    with (
        tc.tile_pool(name="wp", bufs=1) as wp,
        tc.tile_pool(name="pp", bufs=1) as pp,
        tc.tile_pool(name="psA", bufs=6, space="PSUM") as psA,
        tc.tile_pool(name="psB", bufs=2, space="PSUM") as psB,
        tc.tile_pool(name="dram", bufs=1, space="DRAM") as dramp,
    ):
        ident_sb = wp.tile([128, 128], dt.float32)
        nc.sync.dma_start(ident_sb[:], t["ident"][:])
        ones_col = wp.tile([128, 1], dt.float32)
        nc.vector.memset(ones_col[:], 1.0)
        ones_row = wp.tile([1, 128], dt.float32)
        nc.vector.memset(ones_row[:], 1.0)

        gapT = pp.tile([128, NL], dt.float32)
        _encoder(nc, tc, t, wp, pp, psA, gapT)

        # ---- fc: embT [256, NL] = fcw.T @ gapT + fcb; emb output ----
        fcw_sb = wp.tile([128, 256], dt.float32)
        nc.sync.dma_start(fcw_sb[:], t["fcw"][:])
        fcb_sb = wp.tile([128, 2], dt.float32)
        nc.sync.dma_start(fcb_sb[:], t["fcb"].rearrange("(m p) o -> p (m o)", p=128))
        embT = [pp.tile([128, NL], dt.float32, name=f"embT{m}") for m in range(2)]
        emb_sb = pp.tile([NL, 256], dt.float32)
        ag_in = dramp.tile([256, NL], dt.float32)
        ag_out = dramp.tile([256 * N_CORES, NL], dt.float32, addr_space="Shared")
        for m in range(2):
            ps = psB.tile([128, NL], dt.float32, tag="gps")
            nc.tensor.matmul(ps[:], fcw_sb[:, m * 128:(m + 1) * 128], gapT[:],
                             start=True, stop=True)
            nc.scalar.activation(embT[m][:], ps[:], AF.Identity,
                                 bias=fcb_sb[:, m:m + 1])
            nc.sync.dma_start(ag_in[m * 128:(m + 1) * 128, :], embT[m][:])
            tp = psB.tile([128, 128], dt.float32, tag="gps")
            nc.tensor.transpose(tp[:], embT[m][:], ident_sb[:])
            nc.scalar.activation(emb_sb[:, m * 128:(m + 1) * 128], tp[:], AF.Copy)
        nc.sync.dma_start(t["emb_out"][:], emb_sb[:])

        # ---- AllGather embT ----
        nc.gpsimd.collective_compute(
            "AllGather", ALU.bypass,
            replica_groups=[list(range(N_CORES))],
            ins=[ag_in[:]], outs=[ag_out[:]],
        )

        with (
            tc.tile_pool(name="sp", bufs=1) as sp,
            tc.tile_pool(name="g_evac", bufs=3) as ep,
        ):
            # embT_all as [128, 16*128]; free blocks b = 2r+k
            ebT = sp.tile([128, 16 * 128], dt.float32)
            nc.sync.dma_start(ebT[:], ag_out.rearrange("(b p) c -> p b c", p=128))
            ebTv = ebT.rearrange("p (r k c) -> p r k c", r=8, k=2, c=128)

            # Distances from CENTERED embeddings: e' = e - mean(e). d(i,j) is
            # shift-invariant, and centering removes the ~41x cancellation of
            # sq_i + sq_j - 2*dot (||mean||^2 dominates ||e||^2 here), so our
            # d2 is ~exact. The reference's own uncentered fp32 rounding
            # (~6e-7 d2-level) decides a couple of near-tie kNN boundaries;
            # those rows are irreducible coin flips for any implementation
            # that doesn't replicate Eigen's accumulation bit-for-bit.
            msum = ep.tile([128, 2], dt.float32, tag="msum")
            for k in range(2):
                nc.vector.tensor_reduce(msum[:, k:k + 1], ebTv[:, :, k, :],
                                        axis=mybir.AxisListType.XY, op=ALU.add)
            mean = sp.tile([128, 2], dt.float32)
            nc.scalar.activation(mean[:], msum[:], AF.Copy, scale=1.0 / float(N))

            ebC = sp.tile([128, 16 * 128], dt.float32)
            ebCv = ebC.rearrange("p (r k c) -> p r k c", r=8, k=2, c=128)
            for k in range(2):
                nc.vector.tensor_scalar(ebCv[:, :, k, :], ebTv[:, :, k, :],
                                        mean[:, k:k + 1], None, op0=ALU.subtract)
            esq = sp.tile([128, 16 * 128], dt.float32)
            nc.scalar.activation(esq[:], ebC[:], AF.Square)
            esqv = esq.rearrange("p (r k c) -> p r k c", r=8, k=2, c=128)

            # lhsT for d2: lhs_k = -2 * (embT_local - mean)
            lhs_k = [sp.tile([128, 128], dt.float32, name=f"lhs_k{k}")
                     for k in range(2)]
            for k in range(2):
                nc.vector.tensor_scalar(lhs_k[k][:], embT[k][:], mean[:, k:k + 1],
                                        None, op0=ALU.subtract)
                nc.vector.tensor_scalar(lhs_k[k][:], lhs_k[k][:], -2.0, None,
                                        op0=ALU.mult)
            # local sq row: sum_p (0.5*lhs)^2 == e'^2, via ones-lhsT matmul
            sql_ps = psB.tile([1, 128], dt.float32, tag="gps")
            for k in range(2):
                lsq = ep.tile([128, 128], dt.float32, tag="lsq")
                nc.scalar.activation(lsq[:], lhs_k[k][:], AF.Square, scale=0.5)
                nc.tensor.matmul(sql_ps[:], ones_col[:], lsq[:],
                                 start=(k == 0), stop=(k == 1))
            sql_sb = sp.tile([1, 128], dt.float32)
            nc.scalar.activation(sql_sb[:], sql_ps[:], AF.Copy)

            # sq_all row + ones row (all partition-0: engine APs need
            # 32-aligned base partitions, so no [2, N] stacked tiles)
            ones_N = sp.tile([1, N], dt.float32)
            nc.vector.memset(ones_N[:], 1.0)
            sq_row = sp.tile([1, N], dt.float32)
            for half in range(2):
                sq_ps = psB.tile([1, 512], dt.float32, tag="gps")
                for k in range(2):
                    nc.tensor.matmul(sq_ps[:], ones_col[:],
                                     esqv[:, 4 * half:4 * half + 4, k, :],
                                     start=(k == 0), stop=(k == 1))
                nc.scalar.activation(sq_row[0:1, half * 512:(half + 1) * 512],
                                     sq_ps[:], AF.Copy)

            br_sb = ep.tile([1, N], dt.float32, tag="brsb")
            nc.sync.dma_start(br_sb[:], t["biasrow"][:])

            # ---- d2 accumulation + dist + biased selection ----
            dist_sb = sp.tile([NL, N], dt.float32)
            adj_sb = sp.tile([NL, N], dt.float32)
            negd = sp.tile([NL, N], dt.float32)
            for half in range(2):
                cs = slice(half * 512, (half + 1) * 512)
                ps = psB.tile([128, 512], dt.float32, tag="gps")
                for k in range(2):
                    nc.tensor.matmul(ps[:], lhs_k[k][:],
                                     ebCv[:, 4 * half:4 * half + 4, k, :],
                                     start=(k == 0), stop=False,
                                     skip_group_check=True)
                nc.tensor.matmul(ps[:], ones_row[:], sq_row[0:1, cs],
                                 start=False, stop=False, skip_group_check=True)
                nc.tensor.matmul(ps[:], sql_sb[:], ones_N[0:1, cs],
                                 start=False, stop=True, skip_group_check=True)
                rl = ep.tile([128, 512], dt.float32, tag="rl")
                nc.scalar.activation(rl[:], ps[:], AF.Relu)
                nc.scalar.activation(dist_sb[:, cs], rl[:], AF.Sqrt)
                # tie-break bias accumulates AFTER the dist reads
                nc.tensor.matmul(ps[:], ones_row[:], br_sb[0:1, cs], start=False,
                                 stop=True, skip_group_check=True)
                nc.scalar.activation(negd[:, cs], ps[:], AF.Copy, scale=-1.0)
            nc.sync.dma_start(t["dist_out"][:], dist_sb[:])

            m8 = ep.tile([128, 8], dt.float32, tag="m8")
            nc.vector.max(m8[:], negd[:])
            nc.vector.tensor_scalar(adj_sb[:], negd[:], m8[:, 5:6], None,
                                    op0=ALU.is_ge)
            nc.sync.dma_start(t["adj_out"][:], adj_sb[:])

            # ---- adjT blocks via PE transpose ----
            adjT = [sp.tile([128, 128], dt.float32, name=f"adjT{r}")
                    for r in range(8)]
            for r in range(8):
                tp = psB.tile([128, 128], dt.float32, tag="gps")
                nc.tensor.transpose(tp[:], adj_sb[:, r * 128:(r + 1) * 128],
                                    ident_sb[:])
                nc.scalar.activation(adjT[r][:], tp[:], AF.Copy)

            # ---- GCN weights ----
            gw1_sb = [wp.tile([128, 256], dt.float32, name=f"gw1_sb{k}")
                      for k in range(2)]
            gw2_sb = [wp.tile([128, 256], dt.float32, name=f"gw2_sb{k}")
                      for k in range(2)]
            for k in range(2):
                nc.sync.dma_start(gw1_sb[k][:], t["gw1"][k * 128:(k + 1) * 128, :])
                nc.sync.dma_start(gw2_sb[k][:], t["gw2"][k * 128:(k + 1) * 128, :])
            gb1_sb = wp.tile([128, 2], dt.float32)
            nc.sync.dma_start(gb1_sb[:],
                              t["gb1"].rearrange("(m p) o -> p (m o)", p=128))
            gb2_sb = wp.tile([128, 2], dt.float32)
            nc.sync.dma_start(gb2_sb[:],
                              t["gb2"].rearrange("(m p) o -> p (m o)", p=128))

            # layer 1: hg1[r] = emb_all[block r] @ gw1 (node-major out)
            hg1 = [sp.tile([128, 256], dt.float32, name=f"hg1_{r}")
                   for r in range(8)]
            for r in range(8):
                ps = psB.tile([128, 256], dt.float32, tag="gps")
                for k in range(2):
                    nc.tensor.matmul(ps[:], ebTv[:, r, k, :], gw1_sb[k][:],
                                     start=(k == 0), stop=(k == 1))
                nc.scalar.activation(hg1[r][:], ps[:], AF.Copy)
            x1T = [sp.tile([128, 128], dt.float32, name=f"x1T{m}")
                   for m in range(2)]
            for m in range(2):
                ps = psB.tile([128, 128], dt.float32, tag="gps")
                for r in range(8):
                    nc.tensor.matmul(ps[:], hg1[r][:, m * 128:(m + 1) * 128],
                                     adjT[r][:], start=(r == 0), stop=(r == 7))
                nc.scalar.activation(x1T[m][:], ps[:], AF.Relu,
                                     bias=gb1_sb[:, m:m + 1])

            # AllGather x1T
            ag2_in = dramp.tile([256, NL], dt.float32)
            ag2_out = dramp.tile([256 * N_CORES, NL], dt.float32,
                                 addr_space="Shared")
            for m in range(2):
                nc.sync.dma_start(ag2_in[m * 128:(m + 1) * 128, :], x1T[m][:])
            nc.gpsimd.collective_compute(
                "AllGather", ALU.bypass,
                replica_groups=[list(range(N_CORES))],
                ins=[ag2_in[:]], outs=[ag2_out[:]],
            )
            x1a = sp.tile([128, 16 * 128], dt.float32)
            nc.sync.dma_start(x1a[:], ag2_out.rearrange("(b p) c -> p b c", p=128))
            x1av = x1a.rearrange("p (r k c) -> p r k c", r=8, k=2, c=128)

            # layer 2
            hg2 = [sp.tile([128, 256], dt.float32, name=f"hg2_{r}")
                   for r in range(8)]
            for r in range(8):
                ps = psB.tile([128, 256], dt.float32, tag="gps")
                for k in range(2):
                    nc.tensor.matmul(ps[:], x1av[:, r, k, :], gw2_sb[k][:],
                                     start=(k == 0), stop=(k == 1))
                nc.scalar.activation(hg2[r][:], ps[:], AF.Copy)
            x2T = [sp.tile([128, 128], dt.float32, name=f"x2T{m}")
                   for m in range(2)]
            for m in range(2):
                ps = psB.tile([128, 128], dt.float32, tag="gps")
                for r in range(8):
                    nc.tensor.matmul(ps[:], hg2[r][:, m * 128:(m + 1) * 128],
                                     adjT[r][:], start=(r == 0), stop=(r == 7))
                nc.scalar.activation(x2T[m][:], ps[:], AF.Identity,
                                     bias=gb2_sb[:, m:m + 1])

            # ---- heads ----
            cw_sb = [wp.tile([128, 11], dt.float32, name=f"cw_sb{k}")
                     for k in range(2)]
            for k in range(2):
                nc.sync.dma_start(cw_sb[k][:], t["cwt"][k * 128:(k + 1) * 128, :])
            cb_sb = wp.tile([1, 11], dt.float32)
            nc.sync.dma_start(cb_sb[:], t["cbt"][:])
            ps = psB.tile([128, 11], dt.float32, tag="gps")
            for k in range(2):
                nc.tensor.matmul(ps[:], x2T[k][:], cw_sb[k][:], start=(k == 0),
                                 stop=False, skip_group_check=True)
            nc.tensor.matmul(ps[:], ones_row[:], cb_sb[:], start=False, stop=True,
                             skip_group_check=True)
            logit_sb = ep.tile([128, 11], dt.float32, tag="logit")
            nc.scalar.activation(logit_sb[:], ps[:], AF.Copy)
            nc.sync.dma_start(t["cls_out"][:], logit_sb[:, 0:10])
            nc.sync.dma_start(t["spr_out"][:], logit_sb[:, 10:11])


# ---------------------------------------------------------------------------
# host side
# ---------------------------------------------------------------------------

_NC_CACHE = {}


def _get_nc():
    if "nc" not in _NC_CACHE:
        _NC_CACHE["nc"] = build_nc()
    return _NC_CACHE["nc"]


def _f32(x):
    return np.ascontiguousarray(x, dtype=np.float32)


def _w1rep(w1):
    w = _f32(np.asarray(w1).transpose(2, 3, 1, 0).reshape(27, 64))
    out = np.zeros((128, 64), np.float32)
    for a in range(4):
        out[32 * a:32 * a + 27] = w
    return out


def _quad_im2col(col_core):
    # col_core [27, 64 chunks, 512] -> [128, 16, 512]: chunk 16*(j//4)+4*(j%4)+a
    out = np.zeros((128, 16, 512), np.float32)
    j = np.arange(16)
    for a in range(4):
        chunks = 16 * (j // 4) + 4 * (j % 4) + a
        out[32 * a:32 * a + 27] = col_core[:, chunks, :]
    return out.reshape(128, 8192)


def _w2t(w2):
    return _f32(np.asarray(w2).transpose(2, 3, 1, 0).reshape(9, 64, 128))


def _w2pk(w2):
    w = _w2t(w2)
    return _f32(np.stack([np.concatenate([w[3 + kx], w[6 + kx]], 0)
                          for kx in range(3)]))


def _w2e0(w2):
    w = _w2t(w2)
    return _f32(np.stack([w[kx] for kx in range(3)]))


def prep_in_maps(images, w1, b1, w2, b2, aspp_w, aspp_b, proj_w, proj_b,
                 fc_w, fc_b, gcn1_w, gcn1_b, gcn2_w, gcn2_b, cls_w, cls_b,
                 spr_w, spr_b):
    # conv1 im2col on host: [27(ky,kx,c), 1024, 256]
    xp = np.pad(_f32(images), ((0, 0), (0, 0), (1, 1), (1, 1)))
    s = xp.strides
    v = np.lib.stride_tricks.as_strided(
        xp, (N, 3, 3, 3, 16, 16), (s[0], s[1], s[2], s[3], 2 * s[2], 2 * s[3]))
    col = _f32(v.transpose(2, 3, 1, 0, 4, 5).reshape(27, N, 256))

    vsr = np.float32(1.0) / np.sqrt(np.float32(6.0))
    c6 = np.float32(vsr * vsr)

    shared = dict(
        w1c=_w1rep(w1),
        b1c=_f32(b1).reshape(64, 1),
        w2pk=_w2pk(w2),
        w2e0=_w2e0(w2),
        b2c=_f32(b2).reshape(128, 1),
        awt=_f32(np.asarray(aspp_w).transpose(0, 3, 4, 2, 1).reshape(4, 9, 128, 128)),
        abt=_f32(aspp_b).reshape(4, 128, 1),
        pwt=_f32(np.asarray(proj_w)[:, :, 0, 0].T),
        pbt=_f32(proj_b).reshape(128, 1),
        fcw=_f32(fc_w),
        fcb=_f32(fc_b).reshape(256, 1),
        gw1=_f32(c6 * np.asarray(gcn1_w, dtype=np.float32)),
        gb1=_f32(gcn1_b).reshape(256, 1),
        gw2=_f32(c6 * np.asarray(gcn2_w, dtype=np.float32)),
        gb2=_f32(gcn2_b).reshape(256, 1),
        cwt=_f32(np.concatenate([np.asarray(cls_w), np.asarray(spr_w)], axis=1)),
        cbt=_f32(np.concatenate([np.asarray(cls_b), np.asarray(spr_b)]))[None, :],
        biasrow=_f32(np.arange(N) * EPS_BIAS)[None, :],
        ident=np.eye(128, dtype=np.float32),
    )
    in_maps = []
    for c in range(N_CORES):
        m = dict(shared)
        m["im2col1"] = _quad_im2col(
            col[:, c * NL:(c + 1) * NL, :].reshape(27, 64, 512))
        in_maps.append(m)
    return in_maps


def run_on_device(in_maps):
    nc = _get_nc()
    return run_bass_kernel_spmd(nc, in_maps, core_ids=list(range(N_CORES)))


def kernel(**inputs):
    in_maps = prep_in_maps(**inputs)
    res = run_on_device(in_maps)
    emb = np.concatenate([res.results[c]["emb_out"] for c in range(N_CORES)], 0)
    adj = np.concatenate([res.results[c]["adj_out"] for c in range(N_CORES)], 0)
    dist = np.concatenate([res.results[c]["dist_out"] for c in range(N_CORES)], 0)
    lc = np.concatenate([res.results[c]["cls_out"] for c in range(N_CORES)], 0)
    ls = np.concatenate([res.results[c]["spr_out"][:, 0] for c in range(N_CORES)], 0)
    return emb, adj, dist, lc, ls
